# revision 1
# baseline (speedup 1.0000x reference)
"""Multi-head attention (qkv proj + 2D RoPE + softmax attention + out proj)
for Trainium2, data-parallel over 8 NeuronCores (one batch element per core).

kernel(**inputs) takes FULL inputs (tensor (8,1024,1024), w_qkv (3072,1024),
w_proj (1024,1024)) and returns the FULL output (8,1024,1024).

Per-core dataflow (one batch element):
  phase 1 (fp32r): qT/kT = Wq^T.T @ xT (feature-on-partition, head-major
           rows), RoPE via cos/sin-permuted tables (2 PSUM-read multiplies +
           4 partition-block-swap DMAs + add); v token-major (bf16),
           scattered into a padded "vbuf" whose constant ones/zeros columns
           fuse the softmax denominator into the attention matmul.
  phase 2: per head pair: transposed scores (fp32r, K=64, row-group
           concurrent), exp on ScalarE -> bf16 p tiles (scale=1/8, no max
           subtraction - max score ~7), then a contiguous burst of bf16
           o^T matmuls with fused row sums; reciprocal + K=1 broadcast
           matmuls normalize into resident oT tiles (fp32r).
  phase 3 (fp32r): out = oT.T @ wpT with streamed weights.
"""
import numpy as np
import ml_dtypes

import concourse.bass as bass
import concourse.bacc as bacc
import concourse.mybir as mybir
import concourse.tile as tile
from concourse import bass_utils

F32 = mybir.dt.float32
F32R = mybir.dt.float32r
BF16 = mybir.dt.bfloat16
AF = mybir.ActivationFunctionType

B, N, C = 8, 1024, 1024
H, HD = 16, 64
HEIGHT = WIDTH = 32
FREQ = 10000.0
NT = N // 128      # 8 row tiles
CT = C // 128      # 8 contraction tiles
PAIRS = H // 2     # 8 head pairs
VSLOT = 193        # [v_e 64 | one | one | z62 | v_o 64]


# ---------------------------------------------------------------- host prep
def _thetas():
    half = HD // 2
    ifreq = (1.0 / (FREQ ** (np.arange(half, dtype=np.float32) / np.float32(half)))).astype(np.float32)
    fh = np.arange(HEIGHT, dtype=np.float32)[:, None] * ifreq[None, :]
    fw = np.arange(WIDTH, dtype=np.float32)[:, None] * ifreq[None, :]
    th = np.broadcast_to(fh[:, None, :], (HEIGHT, WIDTH, half))
    tw = np.broadcast_to(fw[None, :, :], (HEIGHT, WIDTH, half))
    return np.concatenate([th, tw], axis=-1).reshape(N, HD)


def _host_tables():
    theta = _thetas()
    cos = np.cos(theta).astype(np.float32)     # (N, HD)
    sin = np.sin(theta).astype(np.float32)
    cosT2 = np.empty((128, N), np.float32)     # row 64p+d = cos(theta[:, d])
    sinP = np.empty((128, N), np.float32)      # see RoPE permuted-multiply
    for p in range(2):
        for d in range(HD):
            cosT2[64 * p + d] = cos[:, d]
            if d < 32:
                sinP[64 * p + d] = sin[:, d + 32]
            else:
                sinP[64 * p + d] = -sin[:, d - 32]
    # consts: [0:64]=1 (even-bcast mask), [64:320]=0, [320:384]=1 (odd mask)
    consts = np.zeros((128, 384), np.float32)
    consts[:, 0:64] = 1.0
    consts[:, 320:384] = 1.0
    # vbuf constant pattern (bf16): ones at cols 64 and 97 of each slot
    vconst = np.zeros((128, VSLOT * PAIRS), ml_dtypes.bfloat16)
    for s in range(PAIRS):
        vconst[:, VSLOT * s + 64] = 1.0   # even rowsum -> psum row 64
        vconst[:, VSLOT * s + 97] = 1.0   # odd rowsum -> psum row 32
    return cosT2, sinP, consts, vconst


def _host_weights(w_qkv, w_proj):
    w3 = np.asarray(w_qkv, np.float32).reshape(H, 3 * HD, C)
    wqT = np.ascontiguousarray(w3[:, 0:HD].reshape(H * HD, C).T)
    wkT = np.ascontiguousarray(w3[:, HD:2 * HD].reshape(H * HD, C).T)
    wvT = np.ascontiguousarray(w3[:, 2 * HD:3 * HD].reshape(H * HD, C).T)
    wpT = np.ascontiguousarray(np.asarray(w_proj, np.float32).T)
    return wqT, wkT, wvT, wpT


# ---------------------------------------------------------------- bass build
def build_kernel(nc):
    xT_d = nc.dram_tensor("xT", [C, N], F32, kind="ExternalInput").ap()
    wqT_d = nc.dram_tensor("wqT", [C, C], F32, kind="ExternalInput").ap()
    wkT_d = nc.dram_tensor("wkT", [C, C], F32, kind="ExternalInput").ap()
    wvT_d = nc.dram_tensor("wvT", [C, C], F32, kind="ExternalInput").ap()
    wpT_d = nc.dram_tensor("wpT", [C, C], F32, kind="ExternalInput").ap()
    cos_d = nc.dram_tensor("cosT2", [128, N], F32, kind="ExternalInput").ap()
    sinp_d = nc.dram_tensor("sinP", [128, N], F32, kind="ExternalInput").ap()
    con_d = nc.dram_tensor("consts", [128, 384], F32, kind="ExternalInput").ap()
    vcon_d = nc.dram_tensor("vconst", [128, VSLOT * PAIRS], BF16,
                            kind="ExternalInput").ap()
    out_d = nc.dram_tensor("out", [N, C], F32, kind="ExternalOutput").ap()

    with tile.TileContext(nc) as tc:
        _body(tc, xT_d, wqT_d, wkT_d, wvT_d, wpT_d, cos_d, sinp_d, con_d,
              vcon_d, out_d)
    return nc


def _body(tc, xT_d, wqT_d, wkT_d, wvT_d, wpT_d, cos_d, sinp_d, con_d,
          vcon_d, out_d):
    nc = tc.nc

    with tc.tile_pool(name="persist", bufs=1) as persist, \
         tc.tile_pool(name="cpool", bufs=1) as cpool:
        qR = [persist.tile([128, N], F32R, tag=f"qR{t}", name=f"qR{t}")
              for t in range(PAIRS)]
        kR = [persist.tile([128, N], F32R, tag=f"kR{t}", name=f"kR{t}")
              for t in range(PAIRS)]
        vbuf = [persist.tile([128, VSLOT * PAIRS], BF16, tag=f"vb{tn}",
                             name=f"vb{tn}") for tn in range(NT)]
        oT = [persist.tile([128, N], F32R, tag=f"oT{t}", name=f"oT{t}")
              for t in range(PAIRS)]
        csb = cpool.tile([128, 384], F32R, tag="csb")
        nc.sync.dma_start(csb[:], con_d[:].bitcast(F32R))

        # PE warm-up: fill the initial input-DMA window with dummy matmuls
        # on the constants tile so the HAM un-throttles before phase 1.
        with tc.tile_pool(name="warm", bufs=1, space="PSUM") as wpsum:
            wt = wpsum.tile([128, 384], F32, tag="warm", name="warm")
            for _ in range(64):
                nc.tensor.matmul(wt[:], csb[:, 0:128], csb[:],
                                 start=True, stop=True)

        # -------------------------------------------- phase 1
        with tc.tile_pool(name="tables", bufs=1) as tables, \
             tc.tile_pool(name="xbuf", bufs=1) as xpool, \
             tc.tile_pool(name="wstream", bufs=4) as wpool, \
             tc.tile_pool(name="rope", bufs=3) as rpool, \
             tc.tile_pool(name="pj", bufs=8, space="PSUM") as ppj:

            xT = [xpool.tile([128, N], F32R, tag=f"xT{kc}", name=f"xT{kc}")
                  for kc in range(CT)]
            for kc in range(CT):
                nc.sync.dma_start(
                    xT[kc][:], xT_d[128 * kc:128 * (kc + 1), :].bitcast(F32R))
            cos_sb = tables.tile([128, N], F32R, tag="cos")
            nc.sync.dma_start(cos_sb[:], cos_d[:].bitcast(F32R))
            sinp_sb = tables.tile([128, N], F32R, tag="sinp")
            nc.sync.dma_start(sinp_sb[:], sinp_d[:].bitcast(F32R))

            def proj_rope(w_d, dest):
                for j in range(2):
                    sl = slice(512 * j, 512 * (j + 1))
                    ps = [ppj.tile([128, 512], F32, tag="pj", name="pj")
                          for _ in range(PAIRS)]
                    for kc in range(CT):
                        w = wpool.tile([128, C], F32R, tag="w")
                        nc.sync.dma_start(
                            w[:], w_d[128 * kc:128 * (kc + 1), :].bitcast(F32R))
                        for t in range(PAIRS):
                            nc.tensor.matmul(
                                ps[t][:], w[:, 128 * t:128 * (t + 1)],
                                xT[kc][:, sl],
                                start=(kc == 0), stop=(kc == CT - 1))
                    for t in range(PAIRS):
                        qraw = rpool.tile([128, 512], F32R, tag="qraw",
                                          name="qraw")
                        nc.scalar.copy(qraw[:], ps[t][:])
                        u = rpool.tile([128, 512], F32R, tag="u")
                        up = rpool.tile([128, 512], F32R, tag="up")
                        nc.vector.tensor_mul(u[:], qraw[:], sinp_sb[:, sl])
                        nc.vector.tensor_mul(dest[t][:, sl], qraw[:],
                                             cos_sb[:, sl])
                        for blk in range(4):
                            s = 32 * ((blk // 2) * 2 + 1 - (blk % 2))
                            d = 32 * blk
                            nc.sync.dma_start(up[d:d + 32, :], u[s:s + 32, :])
                        nc.vector.tensor_add(dest[t][:, sl], dest[t][:, sl],
                                             up[:])

            proj_rope(wqT_d, qR)
            proj_rope(wkT_d, kR)

            # vbuf constant pattern (one DMA per row tile), then v scatter
            for tn in range(NT):
                nc.sync.dma_start(vbuf[tn][:], vcon_d[:])
            for jc in range(2):
                ps = [ppj.tile([128, 512], F32, tag="pj", name="pj")
                      for _ in range(NT)]
                for kc in range(CT):
                    w = wpool.tile([128, C], F32R, tag="w")
                    nc.sync.dma_start(
                        w[:], wvT_d[128 * kc:128 * (kc + 1), :].bitcast(F32R))
                    for tn in range(NT):
                        nc.tensor.matmul(
                            ps[tn][:], xT[kc][:, 128 * tn:128 * (tn + 1)],
                            w[:, 512 * jc:512 * (jc + 1)],
                            start=(kc == 0), stop=(kc == CT - 1))
                for tn in range(NT):
                    vsrc = ps[tn][:].rearrange("p (h c) -> p h c", h=8, c=64)
                    vb = vbuf[tn][:].rearrange("p (s c) -> p s c",
                                               s=PAIRS, c=VSLOT)
                    pv = slice(4 * jc, 4 * jc + 4)
                    nc.any.tensor_copy(vb[:, pv, 0:64], vsrc[:, 0::2])
                    nc.any.tensor_copy(vb[:, pv, 129:193], vsrc[:, 1::2])

        # -------------------------------------------- phase 2
        with tc.tile_pool(name="psc", bufs=2, space="PSUM") as psc, \
             tc.tile_pool(name="poe", bufs=2, space="PSUM") as poe, \
             tc.tile_pool(name="poo", bufs=2, space="PSUM") as poo, \
             tc.tile_pool(name="pp", bufs=14) as ppool, \
             tc.tile_pool(name="ns", bufs=2) as nspool:

            for t in range(PAIRS):
                # scores + exp for the whole pair (p tiles in bf16)
                pts = []
                for i in range(NT):
                    p_e = ppool.tile([128, N], BF16, tag="pe", name="pe")
                    p_o = ppool.tile([128, N], BF16, tag="po", name="po")
                    for par, p_sb in ((0, p_e), (1, p_o)):
                        sc = psc.tile([128, N], F32, tag="sc", name="sc")
                        pr = slice(64 * par, 64 * par + 64)
                        for j in range(2):
                            nc.tensor.matmul(
                                sc[:, 512 * j:512 * (j + 1)],
                                kR[t][pr, 128 * i:128 * (i + 1)],
                                qR[t][pr, 512 * j:512 * (j + 1)],
                                start=True, stop=True)
                        nc.scalar.activation(p_sb[:], sc[:], AF.Exp,
                                             scale=0.125)
                    pts.append((p_e, p_o))
                # contiguous o-matmul bursts per nq chunk, then normalize
                for j in range(2):
                    sl = slice(512 * j, 512 * (j + 1))
                    o_e = poe.tile([65, 512], F32, tag="oe", name="oe")
                    o_o = poo.tile([128, 512], F32, tag="oo", name="oo")
                    for i in range(NT):
                        vb = vbuf[i][:]
                        nc.tensor.matmul(
                            o_e[:], vb[:, VSLOT * t:VSLOT * t + 65],
                            pts[i][0][:, sl],
                            start=(i == 0), stop=(i == NT - 1))
                        nc.tensor.matmul(
                            o_o[:], vb[:, VSLOT * t + 65:VSLOT * (t + 1)],
                            pts[i][1][:, sl],
                            start=(i == 0), stop=(i == NT - 1))
                    # normalize: evacuate denominator rows (ACT), one
                    # reciprocal over rows [0:65] (rows 1:63 unused garbage),
                    # K=1 row-group broadcasts, scale
                    rec_e = nspool.tile([128, 512], F32R, tag="rece", name="rece")
                    rec_o = nspool.tile([128, 512], F32R, tag="reco", name="reco")
                    with nc.allow_low_precision(reason="f32r recip feeds bcast"):
                        nc.vector.reciprocal(rec_e[64:65, :], o_e[64:65, :])
                        nc.vector.reciprocal(rec_o[32:33, :], o_o[32:33, :])
                    s_ps = psc.tile([128, N], F32, tag="sc", name="sps")
                    nc.tensor.matmul(s_ps[:, 0:512], csb[64:65, 0:128],
                                     rec_e[64:65, :], start=True, stop=False,
                                     tile_position=(64, 0))
                    nc.tensor.matmul(s_ps[:, 0:512], csb[32:33, 256:384],
                                     rec_o[32:33, :], start=False, stop=True,
                                     tile_position=(32, 0))
                    s_sb = nspool.tile([128, 512], F32, tag="ssb")
                    nc.any.tensor_copy(s_sb[:], s_ps[:, 0:512])
                    nc.vector.tensor_mul(oT[t][0:64, sl], o_e[0:64, :],
                                         s_sb[0:64, :])
                    nc.vector.tensor_mul(oT[t][64:128, sl], o_o[64:128, :],
                                         s_sb[64:128, :])

        # -------------------------------------------- phase 3
        with tc.tile_pool(name="wp3", bufs=8) as wpool3, \
             tc.tile_pool(name="ob", bufs=3) as opool, \
             tc.tile_pool(name="po3", bufs=8, space="PSUM") as ppo:
            wp = []
            for ct in range(CT):
                w = wpool3.tile([128, C], F32R, tag="wp", name="wp")
                nc.sync.dma_start(
                    w[:], wpT_d[128 * ct:128 * (ct + 1), :].bitcast(F32R))
                wp.append(w)
            for jc in range(2):
                ps = [ppo.tile([128, 512], F32, tag="po", name="po")
                      for _ in range(NT)]
                for ct in range(CT):
                    for tn in range(NT):
                        nc.tensor.matmul(ps[tn][:],
                                         oT[ct][:, 128 * tn:128 * (tn + 1)],
                                         wp[ct][:, 512 * jc:512 * (jc + 1)],
                                         start=(ct == 0), stop=(ct == CT - 1))
                for tn in range(NT):
                    ob = opool.tile([128, 512], F32, tag="ob")
                    nc.any.tensor_copy(ob[:], ps[tn][:])
                    nc.sync.dma_start(
                        out_d[128 * tn:128 * (tn + 1), 512 * jc:512 * (jc + 1)],
                        ob[:])


# ---------------------------------------------------------------- entry
_CACHE = {}


def _get_nc():
    if "nc" not in _CACHE:
        nc = bacc.Bacc("TRN2", target_bir_lowering=False, debug=False,
                       num_devices=B)
        build_kernel(nc)
        nc.compile()
        _CACHE["nc"] = nc
    return _CACHE["nc"]


def make_in_maps(tensor, w_qkv, w_proj):
    tensor = np.asarray(tensor, np.float32)
    wqT, wkT, wvT, wpT = _host_weights(w_qkv, w_proj)
    cosT2, sinP, consts, vconst = _host_tables()
    shared = {"wqT": wqT, "wkT": wkT, "wvT": wvT, "wpT": wpT,
              "cosT2": cosT2, "sinP": sinP, "consts": consts,
              "vconst": vconst}
    in_maps = []
    for i in range(B):
        m = dict(shared)
        m["xT"] = np.ascontiguousarray(tensor[i].T)
        in_maps.append(m)
    return in_maps


def run(tensor, w_qkv, w_proj, trace=False):
    in_maps = make_in_maps(tensor, w_qkv, w_proj)
    nc = _get_nc()
    res = bass_utils.run_bass_kernel_spmd(nc, in_maps, core_ids=list(range(B)),
                                          trace=trace)
    out = np.stack([res.results[i]["out"] for i in range(B)])
    return out, res


def kernel(tensor, w_qkv, w_proj):
    out, _ = run(tensor, w_qkv, w_proj, trace=False)
    return out.astype(np.float32)

